# revision 1
# baseline (speedup 1.0000x reference)
"""Trainium2 Bass kernel for nn_DTFormer (histogram_binning).

Math: for each batch row and each of src/dst lists, count (id,snap)
multiset matches (self and cross), then run the counts through two tiny
MLPs.  Since the MLP output depends only on (self_count, cross_count,
snap) -- integers with tiny range -- the whole MLP pipeline is
precomputed host-side into a lookup table T[32*32*8, 128] from the
params.  The device kernel does the real work: the match counting and a
row-gather of T, data-parallel over the 64 batch rows across 8 cores.

Inputs are packed host-side into a single combined key per element:
v = 8*id + (snap-1) < 16384; equality of v <=> equality of (id, snap).
v is supplied both as int16 (for 2x-mode DVE compares) and f32 (for
per-partition scalar operands); valid = (v >= 8).

Counting layout per batch row: E_xy[j, k] = [v_x[j] == v_y[k]] is
processed in 8 j-tiles of [128 x 1024].  Row sums come free via the
compare op's accum_out; dst-cross counts are column sums of E_sd,
accumulated on the PE with a ones-matmul and round-tripped through DRAM
to transpose the layout.
"""

import sys

for p in ("/opt/trn_rl_repo", "/root/.axon_site/_ro/trn_rl_repo"):
    if p not in sys.path:
        sys.path.insert(0, p)

import numpy as np
from contextlib import ExitStack

import concourse.bass as bass
import concourse.bacc as bacc
import concourse.tile as tile
from concourse import mybir
from concourse.bass_utils import run_bass_kernel_spmd

B, L, S, D = 64, 1024, 8, 128
NCORES = 8
BPC = B // NCORES          # batches per core
NT = L // 128              # j-tiles per row
CMAX = 32                  # count clamp (counts are tiny; 32 is ample)
TROWS = CMAX * CMAX * S    # 8192 table rows
N_ACT_DD = 7               # dd j-tiles handled by PE+ACT (rest on DVE)

F32 = mybir.dt.float32
F16 = mybir.dt.float16
BF16 = mybir.dt.bfloat16
I16 = mybir.dt.int16
I32 = mybir.dt.int32
ALU = mybir.AluOpType
ACTF = mybir.ActivationFunctionType

_NC_CACHE = {}
DEBUG_COUNTS = False
TRACE = False
LAST_RESULTS = {}


def build_table(agg_w1, agg_b1, agg_w2, agg_b2, enc_w1, enc_b1, enc_w2, enc_b2):
    """T[a*CMAX*S + b*S + s] = output row for (self=a, cross=b, snap=s+1)."""
    a = np.arange(CMAX, dtype=np.float64)
    w1 = agg_w1.astype(np.float64)      # [S, D]
    b1 = agg_b1.astype(np.float64)      # [D]
    ha = np.maximum(a[None, :, None] * w1[:, None, :] + b1, 0.0)  # [S, CMAX, D]
    g = 0.5 * (ha[:, :, None, :] + ha[:, None, :, :])             # [S, A, B, D]
    y = g @ agg_w2.astype(np.float64) + agg_b2.astype(np.float64)  # [S, A, B, 2]
    ew1 = enc_w1.astype(np.float64)[0]   # [D]
    eb1 = enc_b1.astype(np.float64)
    h0 = np.maximum(y[..., 0:1] * ew1 + eb1, 0.0)  # [S, A, B, D]
    h1 = np.maximum(y[..., 1:2] * ew1 + eb1, 0.0)
    out = (h0 + h1) @ enc_w2.astype(np.float64) + 2.0 * enc_b2.astype(np.float64)
    out = np.transpose(out, (1, 2, 0, 3)).reshape(TROWS, D)  # [A,B,S,D] flat
    return np.ascontiguousarray(out.astype(np.float32))


def _replicate_ap(row_ap, parts=128):
    """AP that reads a DRAM row [N] replicated across `parts` partitions."""
    return bass.AP(tensor=row_ap.tensor, offset=row_ap.offset,
                   ap=[[0, parts]] + [list(p) for p in row_ap.ap])


def _flush_gathers(nc, tc, drsc, feat, table, feat_t, pend):
    # round-trip keys through DRAM into the wrapped/replicated int16 idx
    # layout dma_gather expects: idxs[16g + i, w] = key[16w + i] for every g.
    # Keys of several lists are concatenated so the 8 wrap DMAs are shared.
    n = len(pend)
    ctx_hp = tc.high_priority()
    ctx_hp.__enter__()
    # keys are written to DRAM in the raw [p, t] tile order (contiguous),
    # and wrap stripes are read contiguously; the induced permutation of
    # gather slots is undone by the store AP: slot (p, q) holds output row
    # j = 8p + q.
    k_scr = drsc.tile([1, n * L], I16, tag="kscr", name="k_scr")
    for q, (b_, x_, kt) in enumerate(pend):
        nc.sync.dma_start(
            out=k_scr[0, q * L:(q + 1) * L].rearrange("(p t) -> p t", t=NT),
            in_=kt[:])
    idxs_sb = feat.tile([128, n, L // 16], I16, tag="idxs", name="idxs_sb")
    wrap_ap = k_scr[0, :].rearrange("(q i w) -> i q w", i=16, w=L // 16)
    for g in range(8):
        nc.sync.dma_start(out=idxs_sb[16 * g:16 * (g + 1), :, :], in_=wrap_ap)
    for q, (b_, x_, kt) in enumerate(pend):
        ft = feat.tile([128, NT, D], F32, tag="ft", name="ft")
        nc.gpsimd.dma_gather(
            out_ap=ft[:], in_ap=table[:],
            idxs_ap=idxs_sb[:, q, :],
            num_idxs=L, num_idxs_reg=L, elem_size=D)
        nc.gpsimd.dma_start(
            out=feat_t[x_][b_, :, :].rearrange("(p q) d -> p q d", q=NT),
            in_=ft[:])
    ctx_hp.__exit__(None, None, None)
    pend.clear()


def build_nc():
    nc = bacc.Bacc("TRN2")
    vsn_d = nc.dram_tensor("vsn", [BPC, 128, 2, 2, NT], F32, kind="ExternalInput")
    # K=8 fp16 operands for the PE distance matmul (dst list), all values
    # integer-exact in fp16 via 7-bit digit split of v and hi/lo square
    # splits: d = (vh_j-vh_k)^2 + (vl_j-vl_k)^2, zero iff v_j == v_k.
    # cols 0:L = j-side lhsT rows, cols L:2L = k-side rhs rows.
    quint_d = nc.dram_tensor("quint_d", [BPC, 8, 2 * L], F16, kind="ExternalInput")
    v_i = {x: nc.dram_tensor(f"vi_{x}", [BPC, L], I16, kind="ExternalInput")
           for x in ("s", "d")}
    table = nc.dram_tensor("table", [TROWS, D], F32, kind="ExternalInput")
    feat_t = {"s": nc.dram_tensor("src_feat", [BPC, L, D], F32, kind="ExternalOutput"),
              "d": nc.dram_tensor("dst_feat", [BPC, L, D], F32, kind="ExternalOutput")}
    dbg = {}
    if DEBUG_COUNTS:
        for k in ("ss", "sd", "dd", "ds"):
            dbg[k] = nc.dram_tensor("dbg_" + k, [BPC, 128, NT], F32,
                                    kind="ExternalOutput")

    with tile.TileContext(nc) as tc, ExitStack() as ctx:
        small = ctx.enter_context(tc.tile_pool(name="small", bufs=6))
        bcp = ctx.enter_context(tc.tile_pool(name="bcp", bufs=6))
        pbc = ctx.enter_context(tc.tile_pool(name="pbc", bufs=3, space="PSUM"))
        scr = ctx.enter_context(tc.tile_pool(name="scr", bufs=8))
        feat = ctx.enter_context(tc.tile_pool(name="feat", bufs=8))
        ones = ctx.enter_context(tc.tile_pool(name="ones", bufs=1))
        drsc = ctx.enter_context(tc.tile_pool(name="drsc", bufs=8, space="DRAM"))

        ones_col = ones.tile([128, 1], BF16)
        nc.vector.memset(ones_col[:], 1.0)

        pend = []
        # taper flush groups so the final gathers/stores drain quickly
        flush_after = {2: True, 4: True, 6: True, 7: True}

        for b in range(BPC):
            vv = {}       # [128, NT] f32 (j = t*128 + p), per-partition scalars
            snm = {}      # snap-1 f32
            validv = {}
            vb = {}       # [128, L] int16 broadcast of the v row
            vsn_t = small.tile([128, 2, 2, NT], F32, tag="vsn", name="vsn_t")
            nc.sync.dma_start(out=vsn_t[:], in_=vsn_d[b])
            q5 = small.tile([8, 2 * L], F16, tag="q5", name="q5")
            nc.sync.dma_start(out=q5[:], in_=quint_d[b])
            # flush previous batches' gathers after this batch's loads so the
            # in-order DMA sequencer doesn't head-of-line block the loads
            if pend and flush_after.get(b - 1):
                _flush_gathers(nc, tc, drsc, feat, table, feat_t, pend)
            valid2 = small.tile([128, 2, NT], F32, tag="valid", name="valid2")
            nc.vector.tensor_scalar(
                out=valid2[:], in0=vsn_t[:, 0, :, :], scalar1=8.0, scalar2=None,
                op0=ALU.is_ge)
            sn2 = vsn_t[:, 1, :, :]
            for xi, x in enumerate(("s", "d")):
                vv[x] = vsn_t[:, 0, xi, :]
                vbx = bcp.tile([128, L], I16, tag="vb", name="vb")
                nc.gpsimd.dma_start(out=vbx[:], in_=_replicate_ap(v_i[x][b, :]))
                vb[x] = vbx

            # ---- counting ----
            # cnt_self = [ss | dd], cnt_cross = [sd | ds]
            cnt_self = small.tile([128, 2, NT], F32, tag="cnt_self", name="cnt_self")
            cnt_cross = small.tile([128, 2, NT], F32, tag="cnt_cross", name="cnt_cross")
            cnt = {"ss": cnt_self[:, 0, :], "dd": cnt_self[:, 1, :],
                   "sd": cnt_cross[:, 0, :], "ds": cnt_cross[:, 1, :]}
            for t in range(NT):
                # DVE: src-self
                o = scr.tile([128, L], BF16, tag="scr_v", name="o")
                nc.vector.tensor_scalar(
                    out=o[:], in0=vb["s"][:], scalar1=vv["s"][:, t:t + 1],
                    scalar2=0.0, op0=ALU.is_equal, op1=ALU.add,
                    accum_out=cnt["ss"][:, t:t + 1])
                # DVE: src-cross
                osd = scr.tile([128, L], BF16, tag="scr_sd", name="osd")
                nc.vector.tensor_scalar(
                    out=osd[:], in0=vb["d"][:], scalar1=vv["s"][:, t:t + 1],
                    scalar2=0.0, op0=ALU.is_equal, op1=ALU.add,
                    accum_out=cnt["sd"][:, t:t + 1])
                # DVE: dst-cross
                ods = scr.tile([128, L], BF16, tag="scr_sd", name="ods")
                nc.vector.tensor_scalar(
                    out=ods[:], in0=vb["s"][:], scalar1=vv["d"][:, t:t + 1],
                    scalar2=0.0, op0=ALU.is_equal, op1=ALU.add,
                    accum_out=cnt["ds"][:, t:t + 1])
                # dst-self: first N_ACT_DD tiles on ACT (square + relu(1-x)),
                # the rest on DVE
                if t < N_ACT_DD:
                    # PE computes d = (id_j-id_k)^2 + (sn_j-sn_k)^2; ACT does
                    # relu(1-d) with row-sum accumulation in one pass.
                    d_ps = pbc.tile([128, L], F32, space="PSUM", tag="dps",
                                    name="d_ps")
                    for h in range(2):
                        nc.tensor.matmul(
                            out=d_ps[:, h * 512:(h + 1) * 512],
                            lhsT=q5[:, t * 128:(t + 1) * 128],
                            rhs=q5[:, L + h * 512:L + (h + 1) * 512],
                            start=True, stop=True)
                    o2 = scr.tile([128, L], BF16, tag="scr_a", name="o2")
                    nc.scalar.activation(
                        out=o2[:], in_=d_ps[:], func=ACTF.Relu,
                        bias=1.0, scale=-1.0,
                        accum_out=cnt["dd"][:, t:t + 1])
                else:
                    o3 = scr.tile([128, L], BF16, tag="scr_v", name="o3")
                    nc.vector.tensor_scalar(
                        out=o3[:], in0=vb["d"][:], scalar1=vv["d"][:, t:t + 1],
                        scalar2=0.0, op0=ALU.is_equal, op1=ALU.add,
                        accum_out=cnt["dd"][:, t:t + 1])
            if DEBUG_COUNTS:
                for k in ("ss", "sd", "dd", "ds"):
                    nc.sync.dma_start(out=dbg[k][b], in_=cnt[k][:])

            # ---- table keys:  key = a*CMAX*S + b*S + (sn-1) ----
            a2 = small.tile([128, 2, NT], F32, tag="ka", name="a2")
            nc.vector.tensor_scalar(
                out=a2[:], in0=cnt_self[:], scalar1=float(CMAX - 1),
                scalar2=None, op0=ALU.min)
            nc.vector.tensor_tensor(
                out=a2[:], in0=a2[:], in1=valid2[:], op=ALU.mult)
            b2 = small.tile([128, 2, NT], F32, tag="kb", name="b2")
            nc.vector.tensor_scalar(
                out=b2[:], in0=cnt_cross[:], scalar1=float(CMAX - 1),
                scalar2=None, op0=ALU.min)
            nc.vector.tensor_tensor(
                out=b2[:], in0=b2[:], in1=valid2[:], op=ALU.mult)
            key2 = small.tile([128, 2, NT], F32, tag="key", name="key2")
            nc.vector.scalar_tensor_tensor(
                out=key2[:], in0=a2[:], scalar=float(CMAX * S), in1=sn2,
                op0=ALU.mult, op1=ALU.add)
            nc.vector.scalar_tensor_tensor(
                out=key2[:], in0=b2[:], scalar=float(S), in1=key2[:],
                op0=ALU.mult, op1=ALU.add)
            for xi, x in enumerate(("s", "d")):
                keyi = small.tile([128, NT], I16, tag="keyi", name="keyi")
                nc.vector.tensor_copy(out=keyi[:], in_=key2[:, xi, :])
                pend.append((b, x, keyi))

            if b == BPC - 1:
                _flush_gathers(nc, tc, drsc, feat, table, feat_t, pend)
    nc.compile()
    return nc


def kernel(src_padded_nodes_neighbor_ids, dst_padded_nodes_neighbor_ids,
           src_padded_nodes_snapshots, dst_padded_nodes_snapshots,
           num_snapshots,
           agg_w1, agg_b1, agg_w2, agg_b2, enc_w1, enc_b1, enc_w2, enc_b2):
    tab = build_table(np.asarray(agg_w1), np.asarray(agg_b1),
                      np.asarray(agg_w2), np.asarray(agg_b2),
                      np.asarray(enc_w1), np.asarray(enc_b1),
                      np.asarray(enc_w2), np.asarray(enc_b2))

    if "nc" not in _NC_CACHE:
        _NC_CACHE["nc"] = build_nc()
    nc = _NC_CACHE["nc"]

    ids = {"s": np.asarray(src_padded_nodes_neighbor_ids).astype(np.int64),
           "d": np.asarray(dst_padded_nodes_neighbor_ids).astype(np.int64)}
    sn = {"s": np.asarray(src_padded_nodes_snapshots).astype(np.int64),
          "d": np.asarray(dst_padded_nodes_snapshots).astype(np.int64)}
    v = {x: ids[x] * 8 + (sn[x] - 1) for x in ("s", "d")}

    in_maps = []
    for c in range(NCORES):
        sl = slice(c * BPC, (c + 1) * BPC)
        m = {"table": tab}
        # vsn[b, p, c(v/sn), x(s/d), t]
        vs = np.stack([np.stack([v["s"][sl], v["d"][sl]], axis=1),
                       np.stack([sn["s"][sl] - 1, sn["d"][sl] - 1], axis=1)],
                      axis=1).astype(np.float32)          # [BPC, 2, 2, L]
        vs = vs.reshape(-1, 2, 2, NT, 128).transpose(0, 4, 1, 2, 3)
        m["vsn"] = np.ascontiguousarray(vs)
        vd = v["d"][sl]
        vh = (vd >> 7).astype(np.float64)
        vl = (vd & 127).astype(np.float64)

        def split16(s):
            hi = s.astype(np.float16)
            lo = (s - hi.astype(np.float64)).astype(np.float16)
            return hi, lo

        vh2hi, vh2lo = split16(vh * vh)
        vl2hi, vl2lo = split16(vl * vl)
        k2hi, k2lo = split16(vh * vh + vl * vl)
        one = np.ones_like(vh, dtype=np.float16)
        f16 = np.float16
        qj = np.stack([vh2hi, vh2lo, f16(vh), vl2hi, vl2lo, f16(vl),
                       one, one], axis=1)
        qk = np.stack([one, one, f16(-2.0 * vh), one, one, f16(-2.0 * vl),
                       k2hi, k2lo], axis=1)
        m["quint_d"] = np.ascontiguousarray(
            np.concatenate([qj, qk], axis=2).astype(np.float16))
        for x in ("s", "d"):
            m[f"vi_{x}"] = np.ascontiguousarray(v[x][sl].astype(np.int16))
        in_maps.append(m)
    res = run_bass_kernel_spmd(nc, in_maps, core_ids=list(range(NCORES)),
                               trace=TRACE)
    LAST_RESULTS["res"] = res
    src_feat = np.concatenate([r["src_feat"] for r in res.results], axis=0)
    dst_feat = np.concatenate([r["dst_feat"] for r in res.results], axis=0)
    return (src_feat, dst_feat)

